# revision 19
# baseline (speedup 1.0000x reference)
"""MultiHeadAttention kernel for 8 Trainium2 NeuronCores (v4).

Reference semantics (direct reshape to [B, H, T, hs] makes "heads" contiguous
256-row blocks of Y.reshape(1536, 64) where Y = x[b] @ W):

    k = (x @ Wk).reshape(B, H, T, hs); q, v likewise
    wei = softmax(mask(q @ k^T * C**-0.5))        (causal over chunk index)
    out = (wei @ v).reshape(B, T, C) @ Wp + bp

Sharding: data-parallel over batch - 16 batches per core, weights replicated,
no collectives.

v4 changes vs v3 (265 us):
  * ALL matmul operands bf16.  v3 kept x/Wq/Wk/zq/zk in f32r; the trace
    showed every fp32-mode LDWEIGHTS serializes with its MATMUL (the
    fp32_mode=HIGH weight path cannot use the background weight buffer),
    costing ~LDW+stream per matmul and dropping PE array duty cycle enough
    that HAM throttled the PE to 1.2 GHz for ~half the kernel.  bf16
    weights load via FWL and hide completely: matmuls run at pure
    streaming cost and the array stays warm.
  * zq/zk are still evicted strided (head-major 6t+j layout) in f32, then
    ONE contiguous cast per pair produces bf16 copies for the S matmuls
    (strided bf16 evictions would pay a read-modify-write penalty).
  * S/exp/PV-evict run on head PAIRS: st0 for heads (2i, 2i+1) share one
    [128, 512] PSUM tile and one exp; st1 shrinks to N=128 per head (bf16
    runs full rate at any N; t<128 of s-half-1 is fully masked); po pairs
    share a [65, 512] PSUM tile and one eviction.
  * softmax denominators: the 6 per-head denominator rows live in one
    [65, 1536] o65 tile -> ONE gather DMA per batch into a per-pair
    [6, 512] tile, ONE reciprocal per batch pair.
  * normalize: bc matmuls emit [64, 512] head-pair recip tiles (evicted to
    SBUF), so the 12 [64,128] muls become 6 [64,256] muls.
  * bias folded into the projection as a rank-1 (K=1) ones x bp matmul;
    output DMA'd straight from PSUM (no zo eviction, no bias add).
  * causal masks are [128, 2*128] paired muls on GpSimd (SBUF-only engine);
    everything touching PSUM is split between Scalar and DVE to keep both
    near but below the PE's ~6.6 us/batch.
"""

import sys

if "/opt/trn_rl_repo" not in sys.path:
    sys.path.insert(0, "/opt/trn_rl_repo")

import numpy as np
import ml_dtypes

import concourse.bass as bass
import concourse.mybir as mybir
import concourse.tile as tile
from concourse import bacc
from concourse.bass_utils import run_bass_kernel_spmd

F32 = mybir.dt.float32
F32R = mybir.dt.float32r
BF16 = mybir.dt.bfloat16
Exp = mybir.ActivationFunctionType.Exp

N_CORES = 8
B, T, C = 128, 256, 384
H, HS = 6, 64
NB = B // N_CORES          # batches per core
NP = NB // 2               # batch pairs per core
SCALE = C ** (-0.5)


def build_program(trace_sim=False, sim_init=False):
    nc = bacc.Bacc("TRN2", target_bir_lowering=False, debug=False)

    xT_d = nc.dram_tensor("xT", [NB, 3, 128, T], BF16, kind="ExternalInput")
    wq_d = nc.dram_tensor("wq", [C, C], BF16, kind="ExternalInput")
    wk_d = nc.dram_tensor("wk", [C, C], BF16, kind="ExternalInput")
    wv_d = nc.dram_tensor("wv", [C, C], BF16, kind="ExternalInput")
    # Wp pre-packed on host: wp2[64*par + d, jp, c] = Wp[(2*jp+par)*64 + d, c]
    wp_d = nc.dram_tensor("wp2", [128, 3, C], BF16, kind="ExternalInput")
    bpr_d = nc.dram_tensor("bpr", [1, C], BF16, kind="ExternalInput")
    # tri2 = [tri | tri] so one mul masks a head pair
    tri_d = nc.dram_tensor("tri2", [128, 256], BF16, kind="ExternalInput")
    # sel1[h] rows: sel1_d[k, 64*h + d] = (k == h): one-hot per head
    sel_d = nc.dram_tensor("sel1", [6, 6 * HS], BF16, kind="ExternalInput")
    vsc_d = nc.dram_tensor("vsc", [NB, T, C], BF16)
    rsd_d = nc.dram_tensor("rsd", [NB, 6 * T], BF16)
    out_d = nc.dram_tensor("out", [NB, T, C], F32, kind="ExternalOutput")

    with tile.TileContext(nc, trace_sim=trace_sim) as tc:
        with (
            tc.tile_pool(name="const", bufs=1) as cst,
            tc.tile_pool(name="xt", bufs=3) as xtp,
            tc.tile_pool(name="zqk", bufs=2) as zqkp,
            tc.tile_pool(name="zbf", bufs=2) as zbfp,
            tc.tile_pool(name="yv", bufs=4) as yvp,
            tc.tile_pool(name="vsb", bufs=3) as vp,
            tc.tile_pool(name="ee", bufs=3) as ep,
            tc.tile_pool(name="o65", bufs=3) as op65,
            tc.tile_pool(name="rs", bufs=2) as rsp,
            tc.tile_pool(name="oct", bufs=2) as octp,
            tc.tile_pool(name="zo", bufs=3) as zop,
            tc.tile_pool(name="psG", bufs=2, space="PSUM") as psG,
            tc.tile_pool(name="psA", bufs=4, space="PSUM") as psA,
            tc.tile_pool(name="psZ", bufs=2, space="PSUM") as psZ,
        ):
            # ---- constants (Q/K weights stream first so the prologue
            # GEMMs can start while the rest of the constants load) ----
            wq_sb = cst.tile([128, 3, C], BF16, tag="wq")
            wk_sb = cst.tile([128, 3, C], BF16, tag="wk")
            wv_sb = cst.tile([128, 3, C], BF16, tag="wv")
            nc.sync.dma_start(
                wq_sb[:], wq_d.rearrange("(k p) c -> p k c", p=128))
            wp_sb = cst.tile([128, 3, C], BF16, tag="wp")
            bpr = cst.tile([1, C], BF16, tag="bpr")
            ones1 = cst.tile([1, 128], BF16, tag="ones1")
            tri2 = cst.tile([128, 256], BF16, tag="tri2")
            sel1 = cst.tile([6, 6 * HS], BF16, tag="sel1")

            def load_tail_consts():
                nc.sync.dma_start(
                    wv_sb[:], wv_d.rearrange("(k p) c -> p k c", p=128))
                nc.sync.dma_start(wp_sb[:], wp_d[:])
                nc.sync.dma_start(bpr[:], bpr_d[:])
                nc.sync.dma_start(tri2[:], tri_d[:])
                nc.sync.dma_start(sel1[:], sel_d[:])
                nc.vector.memset(ones1[:], 1.0)

            # Pre-zero the e1 ring (left halves stay zero forever: exp only
            # writes cols 128:256 and 384:512) and pre-set the V ones
            # columns per slot.
            for _ in range(3):
                e1i = ep.tile([128, 2 * T], BF16, tag="e1")
                nc.vector.memset(e1i[:], 0.0)
            for _ in range(3):
                v_ini = vp.tile([128, 12 * (HS + 1)], BF16, tag="vsb")
                if sim_init:
                    nc.vector.memset(v_ini[:], 0.0)
                nc.vector.memset(
                    v_ini[:].rearrange("p (g d) -> p g d", d=HS + 1)[:, :, HS], 1.0)

            # ---------------- pipeline building blocks ----------------
            xts = {}
            zqks = {}
            zbfs = {}
            vsbs = {}
            rss = {}
            recips = {}

            def load_xt(p):
                xt = xtp.tile([128, 3, 2 * T], BF16, tag="xt")
                for n in range(2):
                    nc.sync.dma_start(
                        xt[:].rearrange("p k (n t) -> p k n t", n=2)[:, :, n, :],
                        xT_d[2 * p + n].rearrange("k p t -> p k t"))
                xts[p] = xt

            def qk_group(p, wi, m):
                # one (weight, m-tile) slice of a batch pair's Q/K GEMMs
                if p not in zqks:
                    zq2 = zqkp.tile([64, 12 * T], F32R, tag="zq")
                    zk2 = zqkp.tile([64, 12 * T], F32R, tag="zk")
                    if sim_init:
                        nc.vector.memset(zq2[:], 0.0)
                        nc.vector.memset(zk2[:], 0.0)
                    zqks[p] = (zq2, zk2)
                xt = xts[p]
                wsb = (wq_sb, wk_sb)[wi]
                z2 = zqks[p][wi]
                pq = psG.tile([128, 2 * T], F32, tag="gemm")
                for k in range(3):
                    nc.tensor.matmul(
                        pq[:],
                        wsb[:, k, m * 128:(m + 1) * 128],
                        xt[:, k, :],
                        start=(k == 0), stop=(k == 2),
                    )
                # eviction: [d, bb*1536 + 6t + j] for bb in {0,1}
                nc.vector.tensor_copy(
                    z2[:].rearrange("d (n f) -> d n f", n=2)
                         [:, :, 2 * m:6 * T:6]
                         .rearrange("d n t -> d (n t)"),
                    pq[0:64, :])
                nc.scalar.copy(
                    z2[:].rearrange("d (n f) -> d n f", n=2)
                         [:, :, 2 * m + 1:6 * T:6]
                         .rearrange("d n t -> d (n t)"),
                    pq[64:128, :])

            def z_convert(p, wi):
                # contiguous f32 -> bf16 cast of a pair's zq or zk
                if p not in zbfs:
                    zqb = zbfp.tile([64, 12 * T], BF16, tag="zqb",
                                    name="zqb")
                    zkb = zbfp.tile([64, 12 * T], BF16, tag="zkb",
                                    name="zkb")
                    zbfs[p] = (zqb, zkb)
                src = zqks[p][wi]
                dst = zbfs[p][wi]
                nc.gpsimd.tensor_copy(dst[:], src[:])

            def v_group(b, m):
                xt = xts[b // 2]
                pv = psG.tile([128, C], F32, tag="gemm")
                for k in range(3):
                    nc.tensor.matmul(
                        pv[:],
                        xt[:, k, (b % 2) * T + m * 128:
                           (b % 2) * T + (m + 1) * 128],
                        wv_sb[:, k, :],
                        start=(k == 0), stop=(k == 2),
                    )
                yv = yvp.tile([128, C], BF16, tag="yv")
                nc.scalar.copy(yv[:], pv[:])
                nc.sync.dma_start(vsc_d[b, m * 128:(m + 1) * 128, :], yv[:])
                if m == 1:
                    # V in chunk-row layout [128, 12*(64+1)]; ones cols preset
                    v_sb = vp.tile([128, 12 * (HS + 1)], BF16, tag="vsb")
                    nc.sync.dma_start(
                        v_sb[:].rearrange("p (g d) -> p g d", d=HS + 1)
                              [:, :, 0:HS],
                        vsc_d[b].rearrange("t c -> (t c)")
                                .rearrange("(g p d) -> p g d", p=128, d=64),
                    )
                    vsbs[b] = v_sb

            def v_batch(b):
                v_group(b, 0)
                v_group(b, 1)

            def attention(b, fillers=()):
                zqb, zkb = zbfs[b // 2]
                zq = zqb[:, (b % 2) * 6 * T:(b % 2 + 1) * 6 * T]
                zk = zkb[:, (b % 2) * 6 * T:(b % 2 + 1) * 6 * T]
                v_sb = vsbs.pop(b)
                o65 = op65.tile([HS + 1, 6 * T], BF16, tag="o65")

                def s_pair0(i):
                    # st0 for heads (2i, 2i+1) in one [128, 512] PSUM tile;
                    # mask is upper-tri on each head's cols 0:128
                    st0 = psA.tile([128, 2 * T], F32, tag="att")
                    for hh in range(2):
                        h = 2 * i + hh
                        nc.tensor.matmul(
                            st0[:, hh * T:(hh + 1) * T],
                            zk[:, h * T:h * T + 128],
                            zq[:, h * T:(h + 1) * T],
                            start=True, stop=True)
                    e0 = ep.tile([128, 2 * T], BF16, tag="e0")
                    nc.scalar.activation(e0[:], st0[:], Exp, scale=SCALE)
                    nc.gpsimd.tensor_mul(
                        e0[:].rearrange("p (h t) -> p h t", h=2)[:, :, 0:128],
                        e0[:].rearrange("p (h t) -> p h t", h=2)[:, :, 0:128],
                        tri2[:].rearrange("p (h t) -> p h t", h=2))
                    return e0

                def s_pair1(i):
                    # s-half 1: t<128 fully masked (e1 left halves stay
                    # zero), so only N=128 output cols per head are computed
                    st1 = psA.tile([128, T], F32, tag="att")
                    for hh in range(2):
                        h = 2 * i + hh
                        nc.tensor.matmul(
                            st1[:, hh * 128:(hh + 1) * 128],
                            zk[:, h * T + 128:h * T + 256],
                            zq[:, h * T + 128:(h + 1) * T],
                            start=True, stop=True)
                    e1 = ep.tile([128, 2 * T], BF16, tag="e1")
                    nc.scalar.activation(
                        e1[:].rearrange("p (h t) -> p h t", h=2)
                             [:, :, 128:256],
                        st1[:].rearrange("p (h t) -> p h t", h=2),
                        Exp, scale=SCALE)
                    nc.gpsimd.tensor_mul(
                        e1[:].rearrange("p (h t) -> p h t", h=2)
                             [:, :, 128:256],
                        e1[:].rearrange("p (h t) -> p h t", h=2)
                             [:, :, 128:256],
                        tri2[:].rearrange("p (h t) -> p h t", h=2))
                    return e1

                # S runs one head-pair ahead of PV, interleaved so each
                # pair's exp->mask chain drains before its PV issues
                e0s = [s_pair0(0)]
                e1s = [s_pair1(0)]
                fi = 0
                for i in range(H // 2):
                    if i + 1 < H // 2:
                        e0s.append(s_pair0(i + 1))
                    # PV with fused rowsum (row 64 of each lhsT chunk is
                    # ones); heads 2i and 2i+1 share one [65, 512] PSUM tile
                    po = psA.tile([HS + 1, 2 * T], F32, tag="att")
                    for hh in range(2):
                        h = 2 * i + hh
                        nc.tensor.matmul(
                            po[:, hh * T:(hh + 1) * T],
                            v_sb[:, (2 * h) * (HS + 1):(2 * h + 1) * (HS + 1)],
                            e0s[i][:, hh * T:(hh + 1) * T],
                            start=True, stop=False)
                        nc.tensor.matmul(
                            po[:, hh * T:(hh + 1) * T],
                            v_sb[:, (2 * h + 1) * (HS + 1):
                                 (2 * h + 2) * (HS + 1)],
                            e1s[i][:, hh * T:(hh + 1) * T],
                            start=False, stop=True)
                        if fi < len(fillers):
                            fillers[fi]()
                            fi += 1
                    if i + 1 < H // 2:
                        e1s.append(s_pair1(i + 1))
                    # evict the pair (frees the PSUM slot): rows 0:64 = O^T,
                    # row 64 = softmax denominators
                    nc.scalar.copy(o65[:, 2 * i * T:(2 * i + 2) * T], po[:])
                for f in fillers[fi:]:
                    f()
                # bounce the denominator row through DRAM (a direct
                # SBUF->SBUF partition-split is an illegal DMA pattern)
                nc.sync.dma_start(rsd_d[b], o65[HS:HS + 1, :])
                return o65

            def recip_pair(p):
                rs2b = rsp.tile([6, 2 * T], BF16, tag="rs2b", name="rs2b")
                nc.sync.dma_start(
                    rs2b[:].rearrange("h (n t) -> h n t", n=2),
                    rsd_d[2 * p:2 * p + 2]
                    .rearrange("n (h t) -> h n t", t=T))
                rc = rsp.tile([6, 2 * T], BF16, tag="recip", name="recip")
                with nc.allow_low_precision(reason="softmax denom in bf16"):
                    nc.vector.reciprocal(rc[:], rs2b[:])
                recips[p] = rc

            def norm_pieces(b, o65):
                # normalize + projection, decomposed into five tensor-work
                # chunks so they can double as fillers for the last batches
                rc = recips[b // 2]
                st = {}

                def bc_piece(i):
                    if "ocT2" not in st:
                        st["ocT2"] = octp.tile([128, 768], BF16, tag="ocT2",
                                               name="ocT2")
                    # bcp[d, hh*256 + t] = 1/denom[2i+hh, t] broadcast to 64
                    # rows; the muls read it straight from PSUM
                    bcp = psA.tile([64, 2 * T], F32, tag="att")
                    for hh in range(2):
                        h = 2 * i + hh
                        nc.tensor.matmul(
                            bcp[:, hh * T:(hh + 1) * T],
                            sel1[:, h * HS:(h + 1) * HS],
                            rc[:, (b % 2) * T:(b % 2 + 1) * T],
                            start=True, stop=True)
                    st[i] = bcp

                def mul_piece(i):
                    # ocT2[64*par + d, g//2] = O^T[d, g] / denom[g]; one mul
                    # covers a head pair's 256 ocT2 columns
                    bcp = st.pop(i)
                    for par in range(2):
                        nc.vector.tensor_mul(
                            st["ocT2"][64 * par:64 * (par + 1),
                                       256 * i:256 * (i + 1)],
                            o65[0:HS, 2 * i * T + par:(2 * i + 2) * T:2],
                            bcp[:, par::2])

                def proj_piece(m):
                    pz = psZ.tile([128, C], F32, tag="pz")
                    nc.tensor.matmul(
                        pz[:], ones1[:], bpr[:], start=True, stop=False)
                    for jp in range(3):
                        nc.tensor.matmul(
                            pz[:],
                            st["ocT2"][:, 384 * m + jp:384 * (m + 1):3],
                            wp_sb[:, jp, :],
                            start=False, stop=(jp == 2),
                        )
                    zo = zop.tile([128, C], F32, tag="zo")
                    nc.vector.tensor_copy(zo[:], pz[:])
                    nc.sync.dma_start(out_d[b, m * 128:(m + 1) * 128, :],
                                      zo[:])

                return [lambda: (bc_piece(0), bc_piece(1)),
                        lambda: (mul_piece(0), bc_piece(2)),
                        lambda: (mul_piece(1), mul_piece(2)),
                        lambda: proj_piece(0),
                        lambda: proj_piece(1)]

            def norm_proj(b, o65):
                for f in norm_pieces(b, o65):
                    f()

            # ---------------- software-pipelined schedule ----------------
            # Gap work (next-batch GEMMs) sits between PV(b) and the
            # normalize-dependent bc/proj matmuls so the tensor queue never
            # drains while the denominators' reciprocal round trip runs.
            # norm runs TWO batches behind: recip for pair (2k, 2k+1) is
            # computed right after attention(2k+1), consumed during
            # attention(2k+2)/(2k+3).
            load_xt(0)
            nc.sync.dma_start(
                wk_sb[:], wk_d.rearrange("(k p) c -> p k c", p=128))
            load_tail_consts()
            for wi in range(2):
                for m in range(3):
                    qk_group(0, wi, m)
            z_convert(0, 0)
            z_convert(0, 1)
            v_batch(0)
            v_batch(1)
            load_xt(1)
            hist = {}
            for b in range(NB):
                fillers = []
                if b % 2 == 0:
                    if b // 2 + 1 < NP:
                        p = b // 2 + 1
                        for wi in range(2):
                            for m in range(3):
                                fillers.append(
                                    lambda p=p, wi=wi, m=m: qk_group(p, wi, m))
                else:
                    if b // 2 + 1 < NP:
                        p = b // 2 + 1
                        fillers.append(lambda p=p: z_convert(p, 0))
                        fillers.append(lambda p=p: z_convert(p, 1))
                    for bn in (b + 1, b + 2):
                        if bn < NB:
                            for m in range(2):
                                fillers.append(
                                    lambda bn=bn, m=m: v_group(bn, m))
                    if b // 2 + 2 < NP:
                        fillers.append(lambda p=b // 2 + 2: load_xt(p))
                if b == NB - 1:
                    fillers = [lambda: None] + norm_pieces(b - 2, hist[b - 2])
                cur = attention(b, fillers)
                hist[b] = cur
                if b % 2 == 1:
                    recip_pair(b // 2)
                if b - 2 >= 0 and b != NB - 1:
                    norm_proj(b - 2, hist.pop(b - 2))
            hist.pop(NB - 3)
            norm_proj(NB - 2, hist.pop(NB - 2))
            norm_proj(NB - 1, hist.pop(NB - 1))

    nc.compile()
    return nc


def make_in_maps(x, Wk, Wq, Wv, Wp, bp):
    ut = (np.arange(128)[:, None] <= np.arange(128)[None, :])
    tri = ut.astype(np.float32)
    tri2 = np.concatenate([tri, tri], axis=1)
    sel1 = np.zeros((6, 6 * HS), np.float32)
    for h in range(6):
        sel1[h, HS * h:HS * (h + 1)] = 1.0
    wp2 = (np.asarray(Wp, np.float32).reshape(3, 2, 64, C)
           .transpose(1, 2, 0, 3).reshape(128, 3, C))
    bf = ml_dtypes.bfloat16
    common = dict(
        wq=np.ascontiguousarray(Wq, np.float32).astype(bf),
        wk=np.ascontiguousarray(Wk, np.float32).astype(bf),
        wv=np.ascontiguousarray(Wv, np.float32).astype(bf),
        wp2=np.ascontiguousarray(wp2).astype(bf),
        bpr=np.asarray(bp, np.float32).reshape(1, C).astype(bf),
        tri2=np.ascontiguousarray(tri2).astype(bf),
        sel1=sel1.astype(bf),
    )
    in_maps = []
    for c in range(N_CORES):
        xs = np.asarray(x[c * NB:(c + 1) * NB], np.float32)
        xT = np.ascontiguousarray(xs.transpose(0, 2, 1)).reshape(NB, 3, 128, T)
        in_maps.append(dict(common, xT=xT.astype(bf)))
    return in_maps


_CACHE = {}


def kernel(x, Wk, Wq, Wv, Wp, bp, _trace=False, _tmpdir=None):
    if "nc" not in _CACHE:
        _CACHE["nc"] = build_program()
    nc = _CACHE["nc"]
    in_maps = make_in_maps(x, Wk, Wq, Wv, Wp, bp)
    res = run_bass_kernel_spmd(nc, in_maps, list(range(N_CORES)),
                               trace=_trace, tmpdir=_tmpdir)
    _CACHE["last_results"] = res
    out = np.concatenate([np.asarray(r["out"]) for r in res.results], axis=0)
    return out.reshape(B, T, C).astype(np.float32)


# revision 20
# speedup vs baseline: 1.5332x; 1.5332x over previous
"""MultiHeadAttention kernel for 8 Trainium2 NeuronCores (v4).

Reference semantics (direct reshape to [B, H, T, hs] makes "heads" contiguous
256-row blocks of Y.reshape(1536, 64) where Y = x[b] @ W):

    k = (x @ Wk).reshape(B, H, T, hs); q, v likewise
    wei = softmax(mask(q @ k^T * C**-0.5))        (causal over chunk index)
    out = (wei @ v).reshape(B, T, C) @ Wp + bp

Sharding: data-parallel over batch - 16 batches per core, weights replicated,
no collectives.

v4 changes vs v3 (265 us):
  * ALL matmul operands bf16.  v3 kept x/Wq/Wk/zq/zk in f32r; the trace
    showed every fp32-mode LDWEIGHTS serializes with its MATMUL (the
    fp32_mode=HIGH weight path cannot use the background weight buffer),
    costing ~LDW+stream per matmul and dropping PE array duty cycle enough
    that HAM throttled the PE to 1.2 GHz for ~half the kernel.  bf16
    weights load via FWL and hide completely: matmuls run at pure
    streaming cost and the array stays warm.
  * zq/zk are still evicted strided (head-major 6t+j layout) in f32, then
    ONE contiguous cast per pair produces bf16 copies for the S matmuls
    (strided bf16 evictions would pay a read-modify-write penalty).
  * S/exp/PV-evict run on head PAIRS: st0 for heads (2i, 2i+1) share one
    [128, 512] PSUM tile and one exp; st1 shrinks to N=128 per head (bf16
    runs full rate at any N; t<128 of s-half-1 is fully masked); po pairs
    share a [65, 512] PSUM tile and one eviction.
  * softmax denominators: the 6 per-head denominator rows live in one
    [65, 1536] o65 tile -> ONE gather DMA per batch into a per-pair
    [6, 512] tile, ONE reciprocal per batch pair.
  * normalize: bc matmuls emit [64, 512] head-pair recip tiles (evicted to
    SBUF), so the 12 [64,128] muls become 6 [64,256] muls.
  * bias folded into the projection as a rank-1 (K=1) ones x bp matmul;
    output DMA'd straight from PSUM (no zo eviction, no bias add).
  * causal masks are [128, 2*128] paired muls on GpSimd (SBUF-only engine);
    everything touching PSUM is split between Scalar and DVE to keep both
    near but below the PE's ~6.6 us/batch.
"""

import sys

if "/opt/trn_rl_repo" not in sys.path:
    sys.path.insert(0, "/opt/trn_rl_repo")

import numpy as np
import ml_dtypes

import concourse.bass as bass
import concourse.mybir as mybir
import concourse.tile as tile
from concourse import bacc
from concourse.bass_utils import run_bass_kernel_spmd

F32 = mybir.dt.float32
F32R = mybir.dt.float32r
BF16 = mybir.dt.bfloat16
Exp = mybir.ActivationFunctionType.Exp

N_CORES = 8
B, T, C = 128, 256, 384
H, HS = 6, 64
NB = B // N_CORES          # batches per core
NP = NB // 2               # batch pairs per core
SCALE = C ** (-0.5)


def build_program(trace_sim=False, sim_init=False):
    nc = bacc.Bacc("TRN2", target_bir_lowering=False, debug=False)

    xT_d = nc.dram_tensor("xT", [NB, 3, 128, T], BF16, kind="ExternalInput")
    wq_d = nc.dram_tensor("wq", [C, C], BF16, kind="ExternalInput")
    wk_d = nc.dram_tensor("wk", [C, C], BF16, kind="ExternalInput")
    wv_d = nc.dram_tensor("wv", [C, C], BF16, kind="ExternalInput")
    # Wp pre-packed on host: wp2[64*par + d, jp, c] = Wp[(2*jp+par)*64 + d, c]
    wp_d = nc.dram_tensor("wp2", [128, 3, C], BF16, kind="ExternalInput")
    bpr_d = nc.dram_tensor("bpr", [1, C], BF16, kind="ExternalInput")
    # tri2 = [tri | tri] so one mul masks a head pair
    tri_d = nc.dram_tensor("tri2", [128, 256], BF16, kind="ExternalInput")
    # sel1[h] rows: sel1_d[k, 64*h + d] = (k == h): one-hot per head
    sel_d = nc.dram_tensor("sel1", [6, 6 * HS], BF16, kind="ExternalInput")
    vsc_d = nc.dram_tensor("vsc", [NB, T, C], BF16)
    rsd_d = nc.dram_tensor("rsd", [NB, 6 * T], F32)
    out_d = nc.dram_tensor("out", [NB, T, C], BF16,
                       kind="ExternalOutput")

    with tile.TileContext(nc, trace_sim=trace_sim) as tc:
        with (
            tc.tile_pool(name="const", bufs=1) as cst,
            tc.tile_pool(name="xt", bufs=3) as xtp,
            tc.tile_pool(name="zqk", bufs=2) as zqkp,
            tc.tile_pool(name="zbf", bufs=2) as zbfp,
            tc.tile_pool(name="yv", bufs=4) as yvp,
            tc.tile_pool(name="vsb", bufs=3) as vp,
            tc.tile_pool(name="ee", bufs=3) as ep,
            tc.tile_pool(name="o65", bufs=3) as op65,
            tc.tile_pool(name="rs", bufs=2) as rsp,
            tc.tile_pool(name="oct", bufs=2) as octp,
            tc.tile_pool(name="zo", bufs=3) as zop,
            tc.tile_pool(name="psG", bufs=2, space="PSUM") as psG,
            tc.tile_pool(name="psA", bufs=4, space="PSUM") as psA,
            tc.tile_pool(name="psZ", bufs=2, space="PSUM") as psZ,
        ):
            # ---- constants (Q/K weights stream first so the prologue
            # GEMMs can start while the rest of the constants load) ----
            wq_sb = cst.tile([128, 3, C], BF16, tag="wq")
            wk_sb = cst.tile([128, 3, C], BF16, tag="wk")
            wv_sb = cst.tile([128, 3, C], BF16, tag="wv")
            nc.sync.dma_start(
                wq_sb[:], wq_d.rearrange("(k p) c -> p k c", p=128))
            wp_sb = cst.tile([128, 3, C], BF16, tag="wp")
            bpr = cst.tile([1, C], BF16, tag="bpr")
            ones1 = cst.tile([1, 128], BF16, tag="ones1")
            tri2 = cst.tile([128, 256], BF16, tag="tri2")
            sel1 = cst.tile([6, 6 * HS], BF16, tag="sel1")

            def load_tail_consts():
                nc.sync.dma_start(
                    wv_sb[:], wv_d.rearrange("(k p) c -> p k c", p=128))
                nc.sync.dma_start(wp_sb[:], wp_d[:])
                nc.sync.dma_start(bpr[:], bpr_d[:])
                nc.sync.dma_start(tri2[:], tri_d[:])
                nc.sync.dma_start(sel1[:], sel_d[:])
                nc.vector.memset(ones1[:], 1.0)

            # Pre-zero the e1 ring (left halves stay zero forever: exp only
            # writes cols 128:256 and 384:512) and pre-set the V ones
            # columns per slot.
            for _ in range(3):
                e1i = ep.tile([128, 2 * T], BF16, tag="e1")
                nc.vector.memset(e1i[:], 0.0)
            for _ in range(3):
                v_ini = vp.tile([128, 12 * (HS + 1)], BF16, tag="vsb")
                if sim_init:
                    nc.vector.memset(v_ini[:], 0.0)
                nc.vector.memset(
                    v_ini[:].rearrange("p (g d) -> p g d", d=HS + 1)[:, :, HS], 1.0)

            # ---------------- pipeline building blocks ----------------
            xts = {}
            zqks = {}
            zbfs = {}
            vsbs = {}
            rss = {}
            recips = {}

            def load_xt(p):
                xt = xtp.tile([128, 3, 2 * T], BF16, tag="xt")
                for n in range(2):
                    nc.sync.dma_start(
                        xt[:].rearrange("p k (n t) -> p k n t", n=2)[:, :, n, :],
                        xT_d[2 * p + n].rearrange("k p t -> p k t"))
                xts[p] = xt

            def qk_group(p, wi, m):
                # one (weight, m-tile) slice of a batch pair's Q/K GEMMs
                if p not in zqks:
                    zq2 = zqkp.tile([64, 12 * T], F32R, tag="zq")
                    zk2 = zqkp.tile([64, 12 * T], F32R, tag="zk")
                    if sim_init:
                        nc.vector.memset(zq2[:], 0.0)
                        nc.vector.memset(zk2[:], 0.0)
                    zqks[p] = (zq2, zk2)
                xt = xts[p]
                wsb = (wq_sb, wk_sb)[wi]
                z2 = zqks[p][wi]
                pq = psG.tile([128, 2 * T], F32, tag="gemm")
                for k in range(3):
                    nc.tensor.matmul(
                        pq[:],
                        wsb[:, k, m * 128:(m + 1) * 128],
                        xt[:, k, :],
                        start=(k == 0), stop=(k == 2),
                    )
                # eviction: [d, bb*1536 + 6t + j] for bb in {0,1}
                nc.vector.tensor_copy(
                    z2[:].rearrange("d (n f) -> d n f", n=2)
                         [:, :, 2 * m:6 * T:6]
                         .rearrange("d n t -> d (n t)"),
                    pq[0:64, :])
                nc.scalar.copy(
                    z2[:].rearrange("d (n f) -> d n f", n=2)
                         [:, :, 2 * m + 1:6 * T:6]
                         .rearrange("d n t -> d (n t)"),
                    pq[64:128, :])

            def z_convert(p, wi):
                # contiguous f32 -> bf16 cast of a pair's zq or zk
                if p not in zbfs:
                    zqb = zbfp.tile([64, 12 * T], BF16, tag="zqb",
                                    name="zqb")
                    zkb = zbfp.tile([64, 12 * T], BF16, tag="zkb",
                                    name="zkb")
                    zbfs[p] = (zqb, zkb)
                src = zqks[p][wi]
                dst = zbfs[p][wi]
                if wi == 0:
                    nc.vector.tensor_copy(dst[:], src[:])
                else:
                    nc.scalar.copy(dst[:], src[:])

            def v_group(b, m):
                xt = xts[b // 2]
                pv = psG.tile([128, C], F32, tag="gemm")
                for k in range(3):
                    nc.tensor.matmul(
                        pv[:],
                        xt[:, k, (b % 2) * T + m * 128:
                           (b % 2) * T + (m + 1) * 128],
                        wv_sb[:, k, :],
                        start=(k == 0), stop=(k == 2),
                    )
                yv = yvp.tile([128, C], BF16, tag="yv")
                nc.vector.tensor_copy(yv[:], pv[:])
                nc.sync.dma_start(vsc_d[b, m * 128:(m + 1) * 128, :], yv[:])
                if m == 1:
                    # V in chunk-row layout [128, 12*(64+1)]; ones cols preset
                    v_sb = vp.tile([128, 12 * (HS + 1)], BF16, tag="vsb")
                    nc.sync.dma_start(
                        v_sb[:].rearrange("p (g d) -> p g d", d=HS + 1)
                              [:, :, 0:HS],
                        vsc_d[b].rearrange("t c -> (t c)")
                                .rearrange("(g p d) -> p g d", p=128, d=64),
                    )
                    vsbs[b] = v_sb

            def v_batch(b):
                v_group(b, 0)
                v_group(b, 1)

            def attention(b, fillers=()):
                zqb, zkb = zbfs[b // 2]
                zq = zqb[:, (b % 2) * 6 * T:(b % 2 + 1) * 6 * T]
                zk = zkb[:, (b % 2) * 6 * T:(b % 2 + 1) * 6 * T]
                v_sb = vsbs.pop(b)
                o65 = op65.tile([HS + 1, 6 * T], F32, tag="o65")

                def s_pair0(i):
                    # st0 for heads (2i, 2i+1) in one [128, 512] PSUM tile;
                    # mask is upper-tri on each head's cols 0:128
                    st0 = psA.tile([128, 2 * T], F32, tag="att")
                    for hh in range(2):
                        h = 2 * i + hh
                        nc.tensor.matmul(
                            st0[:, hh * T:(hh + 1) * T],
                            zk[:, h * T:h * T + 128],
                            zq[:, h * T:(h + 1) * T],
                            start=True, stop=True)
                    e0 = ep.tile([128, 2 * T], BF16, tag="e0")
                    nc.scalar.activation(e0[:], st0[:], Exp, scale=SCALE)
                    nc.gpsimd.tensor_mul(
                        e0[:].rearrange("p (h t) -> p h t", h=2)[:, :, 0:128],
                        e0[:].rearrange("p (h t) -> p h t", h=2)[:, :, 0:128],
                        tri2[:].rearrange("p (h t) -> p h t", h=2))
                    return e0

                def s_pair1(i):
                    # s-half 1: t<128 fully masked (e1 left halves stay
                    # zero), so only N=128 output cols per head are computed
                    st1 = psA.tile([128, T], F32, tag="att")
                    for hh in range(2):
                        h = 2 * i + hh
                        nc.tensor.matmul(
                            st1[:, hh * 128:(hh + 1) * 128],
                            zk[:, h * T + 128:h * T + 256],
                            zq[:, h * T + 128:(h + 1) * T],
                            start=True, stop=True)
                    e1 = ep.tile([128, 2 * T], BF16, tag="e1")
                    nc.scalar.activation(
                        e1[:].rearrange("p (h t) -> p h t", h=2)
                             [:, :, 128:256],
                        st1[:].rearrange("p (h t) -> p h t", h=2),
                        Exp, scale=SCALE)
                    nc.gpsimd.tensor_mul(
                        e1[:].rearrange("p (h t) -> p h t", h=2)
                             [:, :, 128:256],
                        e1[:].rearrange("p (h t) -> p h t", h=2)
                             [:, :, 128:256],
                        tri2[:].rearrange("p (h t) -> p h t", h=2))
                    return e1

                # S runs one head-pair ahead of PV, interleaved so each
                # pair's exp->mask chain drains before its PV issues
                e0s = [s_pair0(0)]
                e1s = [s_pair1(0)]
                fi = 0
                for i in range(H // 2):
                    if i + 1 < H // 2:
                        e0s.append(s_pair0(i + 1))
                    # PV with fused rowsum (row 64 of each lhsT chunk is
                    # ones); heads 2i and 2i+1 share one [65, 512] PSUM tile
                    po = psA.tile([HS + 1, 2 * T], F32, tag="att")
                    for hh in range(2):
                        h = 2 * i + hh
                        nc.tensor.matmul(
                            po[:, hh * T:(hh + 1) * T],
                            v_sb[:, (2 * h) * (HS + 1):(2 * h + 1) * (HS + 1)],
                            e0s[i][:, hh * T:(hh + 1) * T],
                            start=True, stop=False)
                        nc.tensor.matmul(
                            po[:, hh * T:(hh + 1) * T],
                            v_sb[:, (2 * h + 1) * (HS + 1):
                                 (2 * h + 2) * (HS + 1)],
                            e1s[i][:, hh * T:(hh + 1) * T],
                            start=False, stop=True)
                        if fi < len(fillers):
                            fillers[fi]()
                            fi += 1
                    if i + 1 < H // 2:
                        e1s.append(s_pair1(i + 1))
                    # evict the pair (frees the PSUM slot): rows 0:64 = O^T,
                    # row 64 = softmax denominators
                    nc.scalar.copy(o65[:, 2 * i * T:(2 * i + 2) * T], po[:])
                for f in fillers[fi:]:
                    f()
                # bounce the denominator row through DRAM (a direct
                # SBUF->SBUF partition-split is an illegal DMA pattern)
                nc.sync.dma_start(rsd_d[b], o65[HS:HS + 1, :])
                return o65

            def recip_pair(p):
                rs2b = rsp.tile([6, 2 * T], F32, tag="rs2b", name="rs2b")
                nc.sync.dma_start(
                    rs2b[:].rearrange("h (n t) -> h n t", n=2),
                    rsd_d[2 * p:2 * p + 2]
                    .rearrange("n (h t) -> h n t", t=T))
                rcf = rsp.tile([6, 2 * T], F32, tag="rcf", name="rcf")
                nc.vector.reciprocal_approx_fast(rcf[:], rs2b[:])
                rc = rsp.tile([6, 2 * T], BF16, tag="recip", name="recip")
                nc.vector.tensor_copy(rc[:], rcf[:])
                recips[p] = rc

            def norm_pieces(b, o65):
                # normalize + projection, decomposed into five tensor-work
                # chunks so they can double as fillers for the last batches
                rc = recips[b // 2]
                st = {}

                def bc_piece(i):
                    if "ocT2" not in st:
                        st["ocT2"] = octp.tile([128, 768], BF16, tag="ocT2",
                                               name="ocT2")
                    # bcp[d, hh*256 + t] = 1/denom[2i+hh, t] broadcast to 64
                    # rows; the muls read it straight from PSUM
                    bcp = psA.tile([64, 2 * T], F32, tag="att")
                    for hh in range(2):
                        h = 2 * i + hh
                        nc.tensor.matmul(
                            bcp[:, hh * T:(hh + 1) * T],
                            sel1[:, h * HS:(h + 1) * HS],
                            rc[:, (b % 2) * T:(b % 2 + 1) * T],
                            start=True, stop=True)
                    st[i] = bcp

                def mul_piece(i):
                    # ocT2[64*par + d, g//2] = O^T[d, g] / denom[g]; one mul
                    # covers a head pair's 256 ocT2 columns
                    bcp = st.pop(i)
                    for par in range(2):
                        nc.vector.tensor_mul(
                            st["ocT2"][64 * par:64 * (par + 1),
                                       256 * i:256 * (i + 1)],
                            o65[0:HS, 2 * i * T + par:(2 * i + 2) * T:2],
                            bcp[:, par::2])

                def proj_piece(m):
                    pz = psZ.tile([128, C], F32, tag="pz")
                    nc.tensor.matmul(
                        pz[:], ones1[:], bpr[:], start=True, stop=False)
                    for jp in range(3):
                        nc.tensor.matmul(
                            pz[:],
                            st["ocT2"][:, 384 * m + jp:384 * (m + 1):3],
                            wp_sb[:, jp, :],
                            start=False, stop=(jp == 2),
                        )
                    zo = zop.tile([128, C], BF16, tag="zo")
                    nc.vector.tensor_copy(zo[:], pz[:])
                    nc.sync.dma_start(out_d[b, m * 128:(m + 1) * 128, :],
                                      zo[:])

                return [lambda: (bc_piece(0), bc_piece(1)),
                        lambda: (mul_piece(0), bc_piece(2)),
                        lambda: (mul_piece(1), mul_piece(2)),
                        lambda: proj_piece(0),
                        lambda: proj_piece(1)]

            def norm_proj(b, o65):
                for f in norm_pieces(b, o65):
                    f()

            # ---------------- software-pipelined schedule ----------------
            # Gap work (next-batch GEMMs) sits between PV(b) and the
            # normalize-dependent bc/proj matmuls so the tensor queue never
            # drains while the denominators' reciprocal round trip runs.
            # norm runs TWO batches behind: recip for pair (2k, 2k+1) is
            # computed right after attention(2k+1), consumed during
            # attention(2k+2)/(2k+3).
            load_xt(0)
            nc.sync.dma_start(
                wk_sb[:], wk_d.rearrange("(k p) c -> p k c", p=128))
            load_tail_consts()
            for wi in range(2):
                for m in range(3):
                    qk_group(0, wi, m)
            z_convert(0, 0)
            z_convert(0, 1)
            v_batch(0)
            v_batch(1)
            load_xt(1)
            hist = {}
            for b in range(NB):
                fillers = []
                if b % 2 == 0:
                    if b // 2 + 1 < NP:
                        p = b // 2 + 1
                        for wi in range(2):
                            for m in range(3):
                                fillers.append(
                                    lambda p=p, wi=wi, m=m: qk_group(p, wi, m))
                else:
                    if b // 2 + 1 < NP:
                        p = b // 2 + 1
                        fillers.append(lambda p=p: z_convert(p, 0))
                        fillers.append(lambda p=p: z_convert(p, 1))
                    for bn in (b + 1, b + 2):
                        if bn < NB:
                            for m in range(2):
                                fillers.append(
                                    lambda bn=bn, m=m: v_group(bn, m))
                    if b // 2 + 2 < NP:
                        fillers.append(lambda p=b // 2 + 2: load_xt(p))
                if b == NB - 1:
                    fillers = [lambda: None] + norm_pieces(b - 2, hist[b - 2])
                cur = attention(b, fillers)
                hist[b] = cur
                if b % 2 == 1:
                    recip_pair(b // 2)
                if b - 2 >= 0 and b != NB - 1:
                    norm_proj(b - 2, hist.pop(b - 2))
            hist.pop(NB - 3)
            norm_proj(NB - 2, hist.pop(NB - 2))
            norm_proj(NB - 1, hist.pop(NB - 1))

    nc.compile()
    return nc


def make_in_maps(x, Wk, Wq, Wv, Wp, bp):
    ut = (np.arange(128)[:, None] <= np.arange(128)[None, :])
    tri = ut.astype(np.float32)
    tri2 = np.concatenate([tri, tri], axis=1)
    sel1 = np.zeros((6, 6 * HS), np.float32)
    for h in range(6):
        sel1[h, HS * h:HS * (h + 1)] = 1.0
    wp2 = (np.asarray(Wp, np.float32).reshape(3, 2, 64, C)
           .transpose(1, 2, 0, 3).reshape(128, 3, C))
    bf = ml_dtypes.bfloat16
    common = dict(
        wq=np.ascontiguousarray(Wq, np.float32).astype(bf),
        wk=np.ascontiguousarray(Wk, np.float32).astype(bf),
        wv=np.ascontiguousarray(Wv, np.float32).astype(bf),
        wp2=np.ascontiguousarray(wp2).astype(bf),
        bpr=np.asarray(bp, np.float32).reshape(1, C).astype(bf),
        tri2=np.ascontiguousarray(tri2).astype(bf),
        sel1=sel1.astype(bf),
    )
    in_maps = []
    for c in range(N_CORES):
        xs = np.asarray(x[c * NB:(c + 1) * NB], np.float32)
        xT = np.ascontiguousarray(xs.transpose(0, 2, 1)).reshape(NB, 3, 128, T)
        in_maps.append(dict(common, xT=xT.astype(bf)))
    return in_maps


_CACHE = {}


def kernel(x, Wk, Wq, Wv, Wp, bp, _trace=False, _tmpdir=None):
    if "nc" not in _CACHE:
        _CACHE["nc"] = build_program()
    nc = _CACHE["nc"]
    in_maps = make_in_maps(x, Wk, Wq, Wv, Wp, bp)
    res = run_bass_kernel_spmd(nc, in_maps, list(range(N_CORES)),
                               trace=_trace, tmpdir=_tmpdir)
    _CACHE["last_results"] = res
    out = np.concatenate([np.asarray(r["out"]) for r in res.results], axis=0)
    return out.reshape(B, T, C).astype(np.float32)
